# revision 2
# baseline (speedup 1.0000x reference)
"""CenterLoss kernel for Trainium2 (8 NeuronCores, data-parallel over batch).

loss = mean_i( clip( ||x_i - centers[labels[i]]||^2, 1e-12, 1e12 ) )

Instead of materializing the [B, C] distance matrix (as the reference does),
we gather the labeled center row per sample with indirect DMA and compute the
squared distance directly: O(B*D) work instead of O(B*C*D).

Sharding: x/labels split into 8 batch shards of 1024 rows; centers replicated.
Each core emits its shard's clipped per-sample distances; the host sums the
8 partials and divides by the global batch (the sanctioned all-reduce).

The datapath runs in fp16 (host casts x/centers once; rel tol is 2e-2 and
fp16 keeps the error ~1e-6 on the mean): halves HBM traffic for both the
x stream and the gathered center rows, and doubles DVE throughput.

Per-core layout (B_loc=1024, P=128 partitions, M=8 row-chunks):
  - sample s lives at (partition p, chunk m) with s = p*8 + m, so the x DMA
    reads 8 KB contiguous per partition (line-rate descriptors).
  - indices ship host-packed as int32 [128, 8] with idx[p, m] = labels[p*8+m];
    the index load is one tiny contiguous DMA and each gather call uses a
    column block directly as its offset AP.
  - 8 indirect gathers of 128 rows each (the SWDGE ucode emits one descriptor
    per partition per call, ~8.7ns/descriptor + ~310ns dispatch per call; this
    chain is the critical path and 128 rows/call is the per-call max).
  - compute per chunk: DVE fp16 subtract, then squared row-sum split between
    ScalarE (fused Square+accum) and DVE (mult + reduce) to balance engines.
"""

import os
import sys

import numpy as np

if "/opt/trn_rl_repo" not in sys.path:
    sys.path.insert(0, "/opt/trn_rl_repo")

_B, _D, _C = 8192, 512, 8000
_N_CORES = 8
_B_LOC = _B // _N_CORES  # 1024 rows per core
_P = 128
_M = _B_LOC // _P  # 8 chunks of 128 rows
_CLAMP_MIN, _CLAMP_MAX = 1e-12, 1e12

_cache: dict = {}


def _build():
    import concourse.bass as bass
    import concourse.tile as tile
    from concourse import bacc, mybir

    nc = bacc.Bacc(
        "TRN2",
        debug=False,
        enable_asserts=False,
        target_bir_lowering=False,
        num_devices=_N_CORES,
        # default 16KB ring fits exactly one 128-desc gather call; a bigger
        # carveout keeps SWDGE descgen off any ring-space stalls.
        dynamic_dma_scratch_size=131072,
    )
    x_d = nc.dram_tensor("x", [_B_LOC, _D], mybir.dt.float16, kind="ExternalInput")
    lab_d = nc.dram_tensor("labels_packed", [_P, _M], mybir.dt.int32, kind="ExternalInput")
    cen_d = nc.dram_tensor("centers", [_C, _D], mybir.dt.float16, kind="ExternalInput")
    out_d = nc.dram_tensor("out", [_P, _M], mybir.dt.float32, kind="ExternalOutput")

    with tile.TileContext(nc) as tc:
        with (
            tc.tile_pool(name="big", bufs=1) as big,
            tc.tile_pool(name="work", bufs=4) as work,
            tc.tile_pool(name="misc", bufs=1) as misc,
        ):
            idx = misc.tile([_P, _M], mybir.dt.int32)
            # idx gates all gather descriptor-gen: keep it first on Sync.
            nc.sync.dma_start(out=idx[:], in_=lab_d.ap())

            xsb = big.tile([_P, _M * _D], mybir.dt.float16)
            nc.sync.dma_start(
                out=xsb[:], in_=x_d.ap().rearrange("(p m) d -> p (m d)", p=_P)
            )

            dist = misc.tile([_P, _M], mybir.dt.float32)
            g = big.tile([_P, _M * _D], mybir.dt.float16)
            g3 = g[:].rearrange("p (m d) -> p m d", d=_D)

            for m in range(_M):
                nc.gpsimd.indirect_dma_start(
                    out=g3[:, m, :],
                    out_offset=None,
                    in_=cen_d.ap(),
                    in_offset=bass.IndirectOffsetOnAxis(
                        ap=idx[:, m : m + 1], axis=0
                    ),
                )

            _DVE_SQ = {4, 6}  # chunks whose square+rowsum runs on DVE
            for m in range(_M):
                diff = work.tile([_P, _D], mybir.dt.float16, tag="diff")
                nc.vector.tensor_tensor(
                    out=diff[:],
                    in0=xsb[:, m * _D : (m + 1) * _D],
                    in1=g[:, m * _D : (m + 1) * _D],
                    op=mybir.AluOpType.subtract,
                )
                if m not in _DVE_SQ:
                    # fused square + row-sum on the scalar engine
                    sq = work.tile([_P, _D], mybir.dt.float16, tag="sq")
                    nc.scalar.activation(
                        out=sq[:],
                        in_=diff[:],
                        func=mybir.ActivationFunctionType.Square,
                        accum_out=dist[:, m : m + 1],
                    )
                else:
                    # balance engines: DVE square + row-sum
                    sq = work.tile([_P, _D], mybir.dt.float32, tag="sqv")
                    nc.vector.tensor_tensor(
                        out=sq[:], in0=diff[:], in1=diff[:], op=mybir.AluOpType.mult
                    )
                    nc.vector.tensor_reduce(
                        out=dist[:, m : m + 1],
                        in_=sq[:],
                        axis=mybir.AxisListType.X,
                        op=mybir.AluOpType.add,
                    )

            # clip both bounds in one DVE op: out = min(max(dist, lo), hi)
            nc.vector.tensor_scalar(
                out=dist[:],
                in0=dist[:],
                scalar1=_CLAMP_MIN,
                scalar2=_CLAMP_MAX,
                op0=mybir.AluOpType.max,
                op1=mybir.AluOpType.min,
            )
            # ship clipped per-sample distances (4 KB); host folds them into
            # the global mean (sum of per-shard sums / global B).
            nc.sync.dma_start(out=out_d.ap()[:, :], in_=dist[:])
    nc.compile()
    return nc


def _pack_labels(labels_shard: np.ndarray) -> np.ndarray:
    """idx[p, m] = labels[p*8 + m], int32 — matches the (p, m) sample layout."""
    return np.ascontiguousarray(labels_shard.reshape(_P, _M).astype(np.int32))


def _run(x, labels, centers, trace=False, **hw_kwargs):
    from concourse import bass_utils

    if "nc" not in _cache:
        _cache["nc"] = _build()
    nc = _cache["nc"]

    x = np.asarray(x)
    labels = np.asarray(labels)
    centers = np.asarray(centers)
    assert x.shape == (_B, _D) and labels.shape == (_B,) and centers.shape == (_C, _D)
    assert labels.min() >= 0 and labels.max() < _C

    cen16 = np.ascontiguousarray(centers.astype(np.float16))
    in_maps = []
    for c in range(_N_CORES):
        sl = slice(c * _B_LOC, (c + 1) * _B_LOC)
        in_maps.append(
            {
                "x": np.ascontiguousarray(x[sl].astype(np.float16)),
                "labels_packed": _pack_labels(labels[sl]),
                "centers": cen16,
            }
        )

    r = bass_utils.run_bass_kernel_spmd(
        nc, in_maps, core_ids=list(range(_N_CORES)), trace=trace, **hw_kwargs
    )
    total = sum(res["out"].astype(np.float64).sum() for res in r.results)
    return np.array(total / _B, dtype=np.float32), r


def kernel(x, labels, centers):
    out, _ = _run(x, labels, centers, trace=False)
    return out
